# revision 10
# baseline (speedup 1.0000x reference)
"""Fused DHCF/LightGCN kernel for 8 Trainium2 NeuronCores.

Math (see reference): three SpMMs (G over the 150k combined node graph,
M1 over users, M2 over items) + ego embedding, averaged by 1/3, then a
row-wise dot over 8192 (user, item) query pairs.

Only the 8192 queried user rows and 8192 queried item rows of the SpMM
outputs are ever needed, so each core computes exactly the 1024 user +
1024 item output rows for its slice of the query batch:

  host:   assign query pairs to (core, tile, row) with a greedy balancer
          that equalizes per-(tile, source-bank) edge counts (minimizes
          block padding); per output row, build the list of (source col,
          val) edges from all three sparse matrices plus the ego edge,
          scale vals by 1/3, group rows into 128-row dest tiles, sort each
          tile's edges by source bank (32768 rows per bank, so indices fit
          int16 for dma_gather), pad each (tile, bank) segment to blocks
          of 128. Embedding table is converted to bf16.
  device: dma_gather 256B bf16 embedding rows per edge block, round-robin
          across the 4 SWDGE queues (each queue's descriptor generation
          runs on its own GPSIMD Q7 core pair, so 4 queues generate
          descriptors concurrently -> ~3.3x on the gather, the baseline
          bottleneck). The per-block one-hot selection matrices (lhsT[e, d]
          = val_e * [dest_e == d], bf16) are fully known on the host, so
          they are precomputed there and streamed in with cheap sequential
          HWDGE DMA instead of being built per block on the DVE (which was
          the next bottleneck) -> PE bf16 matmul accumulates into the dest
          tile's PSUM region (f32) -> finally gamma = rowwise dot of
          user/item tiles, un-permuted on the host.
"""

import sys

sys.path.insert(0, "/opt/trn_rl_repo")

import ml_dtypes
import numpy as np

BF16 = ml_dtypes.bfloat16

NU, NI, D = 100000, 50000, 128
NN = NU + NI
B = 8192
NCORES = 8
QPC = B // NCORES  # queries per core (1024 users + 1024 items)
TILES_PER_KIND = QPC // 128  # 8
NTILES = 2 * TILES_PER_KIND  # 16 dest tiles of 128 rows per core
NTILES_GLOBAL = NCORES * TILES_PER_KIND  # 64 pair-tiles across all cores
BANK = 32768
NBANKS = (NN + BANK - 1) // BANK  # 5
CHUNK_BLOCKS = 8  # blocks (1024 idxs) per dma_gather call; larger calls
                  # overflow the SWDGE descriptor ring and crash the device
NQUEUES = 4       # SWDGE queues (ucode MAX_SWDGE_QUEUES)
THIRD = np.float32(1.0 / 3.0)


# ---------------------------------------------------------------------------
# host-side edge stream construction
# ---------------------------------------------------------------------------

def _sort_by_row(rows, cols, vals):
    order = np.argsort(rows, kind="stable")
    return rows[order], cols[order], vals[order]


def _take_ranges(starts, counts):
    """Concatenate [arange(s, s+c) for s, c in zip(starts, counts)]."""
    total = int(counts.sum())
    if total == 0:
        return np.empty(0, np.int64)
    cum = np.concatenate(([0], np.cumsum(counts)[:-1]))
    return (
        np.repeat(starts.astype(np.int64), counts)
        + np.arange(total, dtype=np.int64)
        - np.repeat(cum, counts)
    )


def _balance_queries(users, items, g_rows, g_cols, m1_rows, m1_cols,
                     m2_rows, m2_cols):
    """Assign each (user, item) query pair to a global slot so that every
    128-row tile sees near-equal per-source-bank edge counts.

    Returns assign[k] = global slot (core = slot // QPC).
    """
    def bank_counts(rows, cols, nrows, col_base=0):
        out = np.zeros((nrows, NBANKS), np.int32)
        np.add.at(out, (rows, (cols + col_base) >> 15), 1)
        return out

    gprof = bank_counts(g_rows, g_cols, NN)
    m1prof = bank_counts(m1_rows, m1_cols, NU)
    m2prof = bank_counts(m2_rows, m2_cols, NI, col_base=NU)

    uprof = gprof[users] + m1prof[users]
    uprof[np.arange(B), users >> 15] += 1  # ego edge
    iprof = gprof[NU + items] + m2prof[items]
    iprof[np.arange(B), (NU + items) >> 15] += 1

    prof = np.concatenate([uprof, iprof], 1).astype(np.float64)
    order = np.argsort(-prof.sum(1))
    loads = np.zeros((NTILES_GLOBAL, 2 * NBANKS), np.float64)
    counts = np.zeros(NTILES_GLOBAL, np.int32)
    assign = np.zeros(B, np.int64)
    for k in order:
        p = prof[k]
        cost = (2.0 * loads * p + p * p).sum(1)
        cost[counts >= 128] = np.inf
        t = int(np.argmin(cost))
        loads[t] += p
        assign[k] = t * 128 + counts[t]
        counts[t] += 1
    # slot s on core c: tile index within core interleaves user tiles first;
    # global tile t = s // 128 maps to core t // TILES_PER_KIND, local tile
    # t % TILES_PER_KIND.  Convert to per-core slot ordering used below.
    return assign


def _tile_edges(keys_g, keys_m, m_col_base, gr, gc, gv, mr, mc, mv):
    """Edges (global col, val/3, dest_local) for one 128-row dest tile.

    keys_g: global node ids for the G matrix lookup, keys_m: local ids for
    the M matrix lookup. Returns cols (int64 global), vals, dest (int64).
    """
    parts_c, parts_v, parts_d = [], [], []
    for keys, (r, c, v), base in ((keys_g, (gr, gc, gv), 0),
                                  (keys_m, (mr, mc, mv), m_col_base)):
        lo = np.searchsorted(r, keys, "left")
        hi = np.searchsorted(r, keys, "right")
        cnt = hi - lo
        take = _take_ranges(lo, cnt)
        parts_c.append(c[take].astype(np.int64) + base)
        parts_v.append(v[take] * THIRD)
        parts_d.append(np.repeat(np.arange(128, dtype=np.int64), cnt))
    # ego edge: col = own global id, val = 1/3
    parts_c.append(keys_g.astype(np.int64))
    parts_v.append(np.full(128, THIRD, np.float32))
    parts_d.append(np.arange(128, dtype=np.int64))
    cols = np.concatenate(parts_c)
    vals = np.concatenate(parts_v).astype(np.float32)
    dest = np.concatenate(parts_d)
    return cols, vals, dest


def preprocess(user_table, item_table, g_vals, m1_vals, m2_vals,
               g_rows, g_cols, m1_rows, m1_cols, m2_rows, m2_cols,
               users, items):
    """Build per-core gather/selection streams.

    Returns (caps, per_core, emb, assign) where assign[k] is the global
    slot each query pair was routed to (for un-permuting gamma).
    """
    users = users.astype(np.int64)
    items = items.astype(np.int64)
    assign = _balance_queries(
        users, items, g_rows.astype(np.int64), g_cols.astype(np.int64),
        m1_rows.astype(np.int64), m1_cols.astype(np.int64),
        m2_rows.astype(np.int64), m2_cols.astype(np.int64))
    # slot_query[s] = original query index routed to slot s
    slot_query = np.empty(B, np.int64)
    slot_query[assign] = np.arange(B)
    users_p = users[slot_query]
    items_p = items[slot_query]

    gr, gc, gv = _sort_by_row(g_rows.astype(np.int64), g_cols, g_vals)
    m1r, m1c, m1v = _sort_by_row(m1_rows.astype(np.int64), m1_cols, m1_vals)
    m2r, m2c, m2v = _sort_by_row(m2_rows.astype(np.int64), m2_cols, m2_vals)

    # per (core, tile): edges sorted by bank, with per-bank counts
    tiles = []  # [core][tile] -> (cols_banked, vals, dest, bank_counts)
    for c in range(NCORES):
        uq = users_p[c * QPC:(c + 1) * QPC]
        iq = items_p[c * QPC:(c + 1) * QPC]
        core_tiles = []
        for t in range(TILES_PER_KIND):
            keys = uq[t * 128:(t + 1) * 128]
            core_tiles.append(_tile_edges(keys, keys, 0, gr, gc, gv, m1r, m1c, m1v))
        for t in range(TILES_PER_KIND):
            keys = iq[t * 128:(t + 1) * 128]
            core_tiles.append(
                _tile_edges(keys + NU, keys, NU, gr, gc, gv, m2r, m2c, m2v))
        tiles.append(core_tiles)

    # bank-sort each tile and count per bank
    binfo = []
    for c in range(NCORES):
        row = []
        for t in range(NTILES):
            cols, vals, dest = tiles[c][t]
            bank = cols >> 15
            order = np.argsort(bank, kind="stable")
            cols, vals, dest, bank = cols[order], vals[order], dest[order], bank[order]
            cnts = np.bincount(bank, minlength=NBANKS)
            row.append((cols, vals, dest, cnts))
        binfo.append(row)

    # shared per-(kind, bank) block capacities = max over cores and tiles
    caps_u = [0] * NBANKS
    caps_i = [0] * NBANKS
    for c in range(NCORES):
        for t in range(NTILES):
            cnts = binfo[c][t][3]
            caps = caps_u if t < TILES_PER_KIND else caps_i
            for b in range(NBANKS):
                caps[b] = max(caps[b], -(-int(cnts[b]) // 128))
    caps = (tuple(caps_u), tuple(caps_i))

    layout = block_layout(caps)
    nblk = layout["nblk"]

    per_core = []
    for c in range(NCORES):
        idx_flat = np.zeros(nblk * 128, np.int16)
        val_flat = np.zeros(nblk * 128, np.float32)
        dest_flat = np.zeros(nblk * 128, np.float32)
        for t in range(NTILES):
            cols, vals, dest, cnts = binfo[c][t]
            off = 0
            for b in range(NBANKS):
                n = int(cnts[b])
                if n:
                    s = layout["seg_start"][(b, t)] * 128
                    idx_flat[s:s + n] = (cols[off:off + n] & (BANK - 1)).astype(np.int16)
                    val_flat[s:s + n] = vals[off:off + n]
                    dest_flat[s:s + n] = dest[off:off + n]
                    off += n
        # wrap indices: element i at [i % 16, i // 16], replicated to all 8
        # 16-partition groups (each GPSIMD core reads its own group).
        idx_w = np.tile(idx_flat.reshape(nblk * 8, 16).T, (8, 1))
        # dense one-hot lhsT stream: block j occupies columns j*128:(j+1)*128,
        # partition = edge slot e, lhsT[e, j*128 + dest_e] = val_e
        lhs = np.zeros((128, nblk * 128), np.float32)
        ee = np.arange(nblk * 128, dtype=np.int64)
        lhs[ee % 128, (ee // 128) * 128 + dest_flat.astype(np.int64)] = val_flat
        per_core.append({
            "idx16": np.ascontiguousarray(idx_w),
            "lhs": np.ascontiguousarray(lhs.astype(BF16)),
        })

    emb = np.ascontiguousarray(
        np.concatenate([user_table, item_table], axis=0).astype(BF16))
    return caps, per_core, emb, assign


def block_layout(caps):
    """Static program structure for given capacities.

    Two waves (user tiles then item tiles) so that at any time each PSUM
    bank hosts exactly one open accumulation group: wave-local tile t
    accumulates in PSUM bank t. Within a wave, blocks are bank-major so
    each dma_gather call stays bank-pure.
    """
    caps_u, caps_i = caps
    blocks = []  # (bank, tile)
    seg_start = {}
    chunks = []  # (bank, first_block, nblocks)
    for w, wcaps in ((0, caps_u), (1, caps_i)):
        for b in range(NBANKS):
            wave_first = len(blocks)
            for t in range(TILES_PER_KIND):
                seg_start[(b, w * TILES_PER_KIND + t)] = len(blocks)
                blocks += [(b, w * TILES_PER_KIND + t)] * wcaps[b]
            nb = len(blocks) - wave_first
            j = 0
            while j < nb:
                n = min(CHUNK_BLOCKS, nb - j)
                chunks.append((b, wave_first + j, n))
                j += n
    nblk = len(blocks)
    # first/last block index per tile (for PSUM start/stop flags)
    first, last = {}, {}
    for i, (b, t) in enumerate(blocks):
        first.setdefault(t, i)
        last[t] = i
    return {"blocks": blocks, "nblk": nblk, "chunks": chunks,
            "seg_start": seg_start, "first": first, "last": last}


def emulate(caps, per_core, emb):
    """Numpy emulation of the device program (validates preprocessing)."""
    layout = block_layout(caps)
    gamma = np.zeros(B, np.float32)
    for c in range(NCORES):
        idx_w = per_core[c]["idx16"]
        nblk = layout["nblk"]
        idx_flat = idx_w[:16, :].T.reshape(-1)  # undo wrap
        lhs = per_core[c]["lhs"].astype(np.float32)  # [128, nblk*128]
        psum = np.zeros((NTILES, 128, D), np.float32)
        for i, (b, t) in enumerate(layout["blocks"]):
            rows = emb[b * BANK + idx_flat[i * 128:(i + 1) * 128].astype(np.int64)]
            onehot = lhs[:, i * 128:(i + 1) * 128]
            psum[t] += onehot.T @ rows.astype(np.float32)
        for j in range(TILES_PER_KIND):
            g = (psum[j] * psum[TILES_PER_KIND + j]).sum(axis=1)
            gamma[c * QPC + j * 128:(c * QPC + (j + 1) * 128)] = g
    return gamma


# ---------------------------------------------------------------------------
# device kernel
# ---------------------------------------------------------------------------

_KERNEL_CACHE = {}
_BUILD_MODE = "full"  # debug knob: full | gather_only | compute_only


def _build_kernel(caps):
    from concourse import bacc, mybir

    from concourse.tile import TileContext

    layout = block_layout(caps)
    nblk = layout["nblk"]

    nc = bacc.Bacc("TRN2", target_bir_lowering=False,
                   num_swdge_queues=NQUEUES)
    f32 = mybir.dt.float32
    bf16 = mybir.dt.bfloat16
    emb_p = nc.declare_dram_parameter("emb", [NN, D], bf16, isOutput=False)
    idx_p = nc.declare_dram_parameter("idx16", [128, nblk * 8], mybir.dt.int16,
                                      isOutput=False)
    lhs_p = nc.declare_dram_parameter("lhs", [128, nblk * 128], bf16,
                                      isOutput=False)
    gamma_p = nc.declare_dram_parameter("gamma", [128, TILES_PER_KIND], f32,
                                        isOutput=True)

    with TileContext(nc) as tc:
        with (
            tc.tile_pool(name="meta", bufs=1) as meta,
            tc.tile_pool(name="gath", bufs=10) as gpool,
            tc.tile_pool(name="lhs", bufs=6) as lpool,
            tc.tile_pool(name="fin", bufs=2) as fpool,
            tc.tile_pool(name="ps", bufs=1, space="PSUM") as pspool,
        ):
            idx_t = meta.tile([128, nblk * 8], mybir.dt.int16, tag="idx")
            gamma_t = meta.tile([128, TILES_PER_KIND], f32, tag="gamma")
            # warm the 4 SWDGE queues with tiny dummy gathers so ring/queue
            # init overlaps the metadata DMA instead of delaying the first
            # real gather; also hoist the num_idxs register shared by all
            # real gathers (one MOVE instead of one per gather call).
            nreg = nc.gpsimd.to_reg(CHUNK_BLOCKS * 128)
            dummy_idx = meta.tile([128, 8], mybir.dt.int16, tag="didx")
            nc.gpsimd.memset(dummy_idx[:], 0)
            for q in range(NQUEUES):
                warm_t = meta.tile([128, 1, D], bf16, tag=f"warm{q}")
                nc.gpsimd.dma_gather(
                    warm_t[:], emb_p[0:BANK, :], dummy_idx[:],
                    128, 128, D, queue_num=q)
            nc.sync.dma_start(out=idx_t[:], in_=idx_p[:])

            # wave-local tile t accumulates in its own PSUM bank t; banks are
            # reused by the item wave once the user wave's result is staged
            # to SBUF (Tile inserts the WAR dependency automatically).
            psum_t = [pspool.tile([128, 128], f32, tag=f"psum{k}",
                                  name=f"psum{k}")
                      for k in range(TILES_PER_KIND)]
            ucopy_t = [fpool.tile([128, 128], f32, tag=f"ucopy{k}",
                                  name=f"ucopy{k}", bufs=1)
                       for k in range(TILES_PER_KIND)]

            for ci, (bank, blk0, n) in enumerate(layout["chunks"]):
                rows_b = min(BANK, NN - bank * BANK)
                g_t = gpool.tile([128, n, D], bf16, tag="gath")
                if _BUILD_MODE != "compute_only":
                    nc.gpsimd.dma_gather(
                        g_t[:],
                        emb_p[bank * BANK:bank * BANK + rows_b, :],
                        idx_t[:, blk0 * 8:(blk0 + n) * 8],
                        n * 128,
                        nreg if n == CHUNK_BLOCKS else n * 128,
                        D,
                        queue_num=ci % NQUEUES,
                    )
                else:
                    nc.vector.memset(g_t[:], 1.0)
                if _BUILD_MODE == "gather_only":
                    continue
                lhs_t = lpool.tile([128, n * 128], bf16, tag="lhs")
                nc.sync.dma_start(
                    out=lhs_t[:], in_=lhs_p[:, blk0 * 128:(blk0 + n) * 128])
                for j in range(n):
                    blk = blk0 + j
                    t = layout["blocks"][blk][1]
                    nc.tensor.matmul(
                        out=psum_t[t % TILES_PER_KIND][:],
                        lhsT=lhs_t[:, j * 128:(j + 1) * 128],
                        rhs=g_t[:, j, :],
                        start=(layout["first"][t] == blk),
                        stop=(layout["last"][t] == blk),
                    )
                    if layout["last"][t] == blk and t < TILES_PER_KIND:
                        # user wave done for this bank: stage to SBUF on the
                        # otherwise-idle ACT engine, freeing the bank for the
                        # item wave.
                        nc.scalar.copy(out=ucopy_t[t][:], in_=psum_t[t][:])

            if _BUILD_MODE == "gather_only":
                nc.vector.memset(gamma_t[:], 0.0)
                for k in range(TILES_PER_KIND):
                    nc.vector.memset(psum_t[k][:], 0.0)
                    nc.vector.memset(ucopy_t[k][:], 0.0)
            for j in range(TILES_PER_KIND):
                prod_t = fpool.tile([128, 128], f32, tag="prod")
                nc.vector.tensor_tensor(
                    out=prod_t[:],
                    in0=ucopy_t[j][:],
                    in1=psum_t[j][:],
                    op=mybir.AluOpType.mult,
                )
                nc.vector.tensor_reduce(
                    out=gamma_t[:, j:j + 1],
                    in_=prod_t[:],
                    axis=mybir.AxisListType.X,
                    op=mybir.AluOpType.add,
                )
            nc.sync.dma_start(out=gamma_p[:], in_=gamma_t[:])

    nc.compile()
    return nc


def get_kernel(caps):
    if caps not in _KERNEL_CACHE:
        _KERNEL_CACHE[caps] = _build_kernel(caps)
    return _KERNEL_CACHE[caps]


def kernel(user_table, item_table, g_vals, m1_vals, m2_vals,
           g_rows, g_cols, m1_rows, m1_cols, m2_rows, m2_cols,
           users, items, _trace=False):
    from concourse.bass_utils import run_bass_kernel_spmd

    caps, per_core, emb, assign = preprocess(
        np.asarray(user_table), np.asarray(item_table), np.asarray(g_vals),
        np.asarray(m1_vals), np.asarray(m2_vals), np.asarray(g_rows),
        np.asarray(g_cols), np.asarray(m1_rows), np.asarray(m1_cols),
        np.asarray(m2_rows), np.asarray(m2_cols), np.asarray(users),
        np.asarray(items))

    nc = get_kernel(caps)
    in_maps = [
        {"emb": emb, **per_core[c]} for c in range(NCORES)
    ]
    res = run_bass_kernel_spmd(nc, in_maps, core_ids=list(range(NCORES)),
                               trace=_trace)
    gamma_slots = np.empty(B, np.float32)
    for c in range(NCORES):
        gamma_slots[c * QPC:(c + 1) * QPC] = res.results[c]["gamma"].T.reshape(-1)
    gamma = gamma_slots[assign]
    if _trace:
        kernel._last_result = res
    return gamma
